# revision 1
# baseline (speedup 1.0000x reference)
"""Trainium2 Bass kernel for nn_Mlp_2_Layer (moe_routing).

Strategy: data-parallel over the batch. Each of the 8 NeuronCores takes
B/8 = 1024 samples and runs them through all D=8 per-domain MLPs.
Training-mode BatchNorm statistics span the full batch, so per-core
partial stats (mean, E[x^2]) are combined with two small AllReduce
collectives. Device pipeline per core:

  1. Embedding gather: 128 indirect-DMA gathers (one per (feature,
     batch-tile); one table row per SBUF partition) from a flattened
     [F*V, E] table, using host-prepared int32 indices f*V + id.
  2. PE-transpose the gathered [batch, feature] tiles into XT
     [512 features, 1024 batch] (feature-major, fp32r-rounded).
  3. L1 stats pass: pre1 = W1 @ X for all domains; bn_stats/bn_aggr
     per 128-row tile; per-core (mean, E[x^2]) -> AllReduce.
  4. L1 apply + L2: relu(s1*pre1+t1) fused on the Scalar engine,
     h2pre = W2 @ a1, bn_stats again -> AllReduce; h2pre spilled to HBM.
  5. Output: a2 = relu(s2*h2pre+t2), dot with W3 via a [128->1] matmul
     (lhsT = W3 column), sigmoid(+b3) -> out[d, b].

Host combines: final[b] = out[domain_id[b], b].

All matmuls run in float32r (TF32-like, full PE rate at N=512).
"""
import sys

for _p in ("/opt/trn_rl_repo", "/root/.axon_site"):
    if _p not in sys.path:
        sys.path.insert(0, _p)

import numpy as np

B, F, E, V = 8192, 16, 32, 100000
D, H1, H2 = 8, 1024, 512
IN = F * E          # 512
EPS = 1e-5
NCORES = 8
BC = B // NCORES    # 1024 samples per core
NBT = BC // 128     # 8 batch tiles per core
P = 128
NT = BC // 512      # 2 n-chunks of 512 per core
K1 = IN // P        # 4 k-tiles for layer 1
M1 = H1 // P        # 8 m-tiles for layer 1
K2 = H1 // P        # 8 k-tiles for layer 2
M2 = H2 // P        # 4 m-tiles for layer 2

PROFILE = False       # test.py sets kernel.PROFILE = True
LAST_EXEC_NS = None   # filled when PROFILE

_NC = None


def _build():
    import concourse.bass as bass
    import concourse.tile as tile
    from concourse import bacc, mybir
    from concourse.masks import make_identity

    f32 = mybir.dt.float32
    f32r = mybir.dt.float32r
    i32 = mybir.dt.int32
    AF = mybir.ActivationFunctionType

    nc = bacc.Bacc(None, target_bir_lowering=False, debug=False)

    tab_d = nc.dram_tensor("tab", [F * V, E], f32, kind="ExternalInput")
    gidx_d = nc.dram_tensor("gidx", [P, NBT * F], i32, kind="ExternalInput")
    w1t_d = nc.dram_tensor("w1t", [D, IN, H1], f32r, kind="ExternalInput")
    w2t_d = nc.dram_tensor("w2t", [D, H1, H2], f32r, kind="ExternalInput")
    g1_d = nc.dram_tensor("g1", [D, H1], f32, kind="ExternalInput")
    be1_d = nc.dram_tensor("be1", [D, H1], f32, kind="ExternalInput")
    g2_d = nc.dram_tensor("g2", [D, H2], f32, kind="ExternalInput")
    be2_d = nc.dram_tensor("be2", [D, H2], f32, kind="ExternalInput")
    w3_d = nc.dram_tensor("w3", [D, H2], f32r, kind="ExternalInput")
    b3_d = nc.dram_tensor("b3", [1, D], f32, kind="ExternalInput")
    out_d = nc.dram_tensor("out", [D, BC], f32, kind="ExternalOutput")

    h2_dram = nc.dram_tensor("h2s", [D, P, M2, BC], f32, kind="Internal")
    cc1_in = nc.dram_tensor("cc1i", [P, 2 * D * M1], f32, kind="Internal")
    cc1_out = nc.dram_tensor("cc1o", [P, 2 * D * M1], f32, kind="Internal",
                             addr_space="Shared")
    cc2_in = nc.dram_tensor("cc2i", [P, 2 * D * M2], f32, kind="Internal")
    cc2_out = nc.dram_tensor("cc2o", [P, 2 * D * M2], f32, kind="Internal",
                             addr_space="Shared")
    RG = [list(range(NCORES))]

    with tile.TileContext(nc) as tc:
        from contextlib import ExitStack
        with ExitStack() as ctx:
            const = ctx.enter_context(tc.tile_pool(name="const", bufs=1))
            gpool = ctx.enter_context(tc.tile_pool(name="gpool", bufs=3))
            xtp = ctx.enter_context(tc.tile_pool(name="xtp", bufs=1))
            wpool = ctx.enter_context(tc.tile_pool(name="wpool", bufs=3))
            a1p = ctx.enter_context(tc.tile_pool(name="a1p", bufs=2))
            h2p = ctx.enter_context(tc.tile_pool(name="h2p", bufs=3))
            stp = ctx.enter_context(tc.tile_pool(name="stp", bufs=1))
            outp = ctx.enter_context(tc.tile_pool(name="outp", bufs=2))
            ps = ctx.enter_context(tc.tile_pool(name="ps", bufs=5, space="PSUM"))
            pst = ctx.enter_context(tc.tile_pool(name="pst", bufs=2, space="PSUM"))
            pso = ctx.enter_context(tc.tile_pool(name="pso", bufs=1, space="PSUM"))

            ident = const.tile([P, P], f32)
            make_identity(nc, ident[:])
            eps_t = const.tile([P, 1], f32)
            nc.vector.memset(eps_t[:], EPS)

            gidx = const.tile([P, NBT * F], i32)
            nc.sync.dma_start(out=gidx[:], in_=gidx_d[:, :])

            # per-partition column layouts of the small per-domain params
            g1c = const.tile([P, D * M1], f32)
            nc.sync.dma_start(out=g1c[:], in_=g1_d[:, :].rearrange(
                "d (m p) -> p (d m)", p=P))
            be1c = const.tile([P, D * M1], f32)
            nc.sync.dma_start(out=be1c[:], in_=be1_d[:, :].rearrange(
                "d (m p) -> p (d m)", p=P))
            g2c = const.tile([P, D * M2], f32)
            nc.sync.dma_start(out=g2c[:], in_=g2_d[:, :].rearrange(
                "d (m p) -> p (d m)", p=P))
            be2c = const.tile([P, D * M2], f32)
            nc.sync.dma_start(out=be2c[:], in_=be2_d[:, :].rearrange(
                "d (m p) -> p (d m)", p=P))
            w3c = const.tile([P, D * M2], f32r)
            nc.sync.dma_start(out=w3c[:], in_=w3_d[:, :].rearrange(
                "d (m p) -> p (d m)", p=P))
            b3r = const.tile([1, D], f32)
            nc.sync.dma_start(out=b3r[:], in_=b3_d[:, :])

            # ---- Phase 1: gather + transpose into XT (feature-major) ----
            xt = [xtp.tile([P, K1, 512], f32r, name=f"xt{i}") for i in range(NT)]
            for t in range(NBT):
                G = gpool.tile([P, F, E], f32, tag="G")
                for f in range(F):
                    nc.gpsimd.indirect_dma_start(
                        out=G[:, f, :],
                        out_offset=None,
                        in_=tab_d[:, :],
                        in_offset=bass.IndirectOffsetOnAxis(
                            ap=gidx[:, t * F + f: t * F + f + 1], axis=0),
                    )
                gflat = G[:].rearrange("p f e -> p (f e)")
                for k in range(K1):
                    tp = pst.tile([P, P], f32, tag="tp")
                    nc.tensor.transpose(
                        out=tp[:], in_=gflat[:, k * P:(k + 1) * P],
                        identity=ident[:])
                    nc.vector.tensor_copy(
                        out=xt[t // 4][:, k, (t % 4) * P:(t % 4 + 1) * P],
                        in_=tp[:])

            # ---- Phase 2: L1 stats pass ----
            st1 = stp.tile([P, D, M1, NT, 6], f32)
            for d in range(D):
                w1 = wpool.tile([P, K1, H1], f32r, tag="w")
                nc.sync.dma_start(
                    out=w1[:], in_=w1t_d[d, :, :].rearrange(
                        "(k p) h -> p k h", p=P))
                for m in range(M1):
                    for nt in range(NT):
                        pm = ps.tile([P, 512], f32, tag="ps")
                        for k in range(K1):
                            nc.tensor.matmul(
                                out=pm[:],
                                lhsT=w1[:, k, m * P:(m + 1) * P],
                                rhs=xt[nt][:, k, :],
                                start=(k == 0), stop=(k == K1 - 1))
                        nc.vector.bn_stats(out=st1[:, d, m, nt, :], in_=pm[:])

            # aggregate per-core, AllReduce (mean, E[x^2]) across cores
            uq1 = stp.tile([P, 2 * D * M1], f32)
            mv1 = stp.tile([P, D, M1, 2], f32)
            for d in range(D):
                for m in range(M1):
                    nc.vector.bn_aggr(out=mv1[:, d, m, :], in_=st1[:, d, m, :, :])
            u1 = uq1[:, 0:D * M1].rearrange("p (d m) -> p d m", d=D)
            q1 = uq1[:, D * M1:].rearrange("p (d m) -> p d m", d=D)
            nc.vector.tensor_copy(out=u1, in_=mv1[:, :, :, 0])
            nc.vector.tensor_mul(out=q1, in0=mv1[:, :, :, 0], in1=mv1[:, :, :, 0])
            nc.vector.tensor_add(out=q1, in0=q1, in1=mv1[:, :, :, 1])
            nc.sync.dma_start(out=cc1_in[:, :], in_=uq1[:])
            import concourse.mybir as mybir_
            nc.gpsimd.collective_compute(
                "AllReduce", mybir_.AluOpType.add, replica_groups=RG,
                ins=[cc1_in[:, :]], outs=[cc1_out[:, :]])
            s_all1 = stp.tile([P, 2 * D * M1], f32)
            nc.sync.dma_start(out=s_all1[:], in_=cc1_out[:, :])

            # s1 = g1 / sqrt(var+eps), t1 = be1 - mean*s1   [P, D*M1]
            s1 = stp.tile([P, D * M1], f32)
            t1 = stp.tile([P, D * M1], f32)
            mean1 = stp.tile([P, D * M1], f32)
            var1 = stp.tile([P, D * M1], f32)
            nc.vector.tensor_scalar_mul(mean1[:], s_all1[:, 0:D * M1], 1.0 / NCORES)
            nc.vector.tensor_scalar_mul(var1[:], s_all1[:, D * M1:], 1.0 / NCORES)
            nc.vector.tensor_mul(out=s1[:], in0=mean1[:], in1=mean1[:])
            nc.vector.tensor_tensor(
                out=var1[:], in0=var1[:], in1=s1[:],
                op=mybir_.AluOpType.subtract)
            nc.scalar.activation(out=var1[:], in_=var1[:], func=AF.Sqrt,
                                 bias=eps_t[:], scale=1.0)
            nc.vector.reciprocal(out=var1[:], in_=var1[:])
            nc.vector.tensor_mul(out=s1[:], in0=g1c[:], in1=var1[:])
            nc.vector.tensor_mul(out=t1[:], in0=mean1[:], in1=s1[:])
            nc.vector.tensor_tensor(
                out=t1[:], in0=be1c[:], in1=t1[:], op=mybir_.AluOpType.subtract)

            # ---- Phase 3: L1 apply + L2 ----
            st2 = stp.tile([P, D, M2, NT, 6], f32)
            for d in range(D):
                w1 = wpool.tile([P, K1, H1], f32r, tag="w")
                nc.sync.dma_start(
                    out=w1[:], in_=w1t_d[d, :, :].rearrange(
                        "(k p) h -> p k h", p=P))
                w2 = wpool.tile([P, K2, H2], f32r, tag="w")
                nc.sync.dma_start(
                    out=w2[:], in_=w2t_d[d, :, :].rearrange(
                        "(k p) h -> p k h", p=P))
                for nt in range(NT):
                    a1 = a1p.tile([P, K2, 512], f32r, tag="a1")
                    for m in range(M1):
                        pm = ps.tile([P, 512], f32, tag="ps")
                        for k in range(K1):
                            nc.tensor.matmul(
                                out=pm[:],
                                lhsT=w1[:, k, m * P:(m + 1) * P],
                                rhs=xt[nt][:, k, :],
                                start=(k == 0), stop=(k == K1 - 1))
                        dm = d * M1 + m
                        nc.scalar.activation(
                            out=a1[:, m, :], in_=pm[:], func=AF.Relu,
                            bias=t1[:, dm:dm + 1], scale=s1[:, dm:dm + 1])
                    for m2 in range(M2):
                        pm2 = ps.tile([P, 512], f32, tag="ps")
                        for k2 in range(K2):
                            nc.tensor.matmul(
                                out=pm2[:],
                                lhsT=w2[:, k2, m2 * P:(m2 + 1) * P],
                                rhs=a1[:, k2, :],
                                start=(k2 == 0), stop=(k2 == K2 - 1))
                        nc.vector.bn_stats(out=st2[:, d, m2, nt, :], in_=pm2[:])
                        h2c = h2p.tile([P, 512], f32, tag="h2c")
                        nc.vector.tensor_copy(out=h2c[:], in_=pm2[:])
                        nc.sync.dma_start(
                            out=h2_dram[d, :, m2, nt * 512:(nt + 1) * 512],
                            in_=h2c[:])

            uq2 = stp.tile([P, 2 * D * M2], f32)
            mv2 = stp.tile([P, D, M2, 2], f32)
            for d in range(D):
                for m2 in range(M2):
                    nc.vector.bn_aggr(out=mv2[:, d, m2, :], in_=st2[:, d, m2, :, :])
            u2 = uq2[:, 0:D * M2].rearrange("p (d m) -> p d m", d=D)
            q2 = uq2[:, D * M2:].rearrange("p (d m) -> p d m", d=D)
            nc.vector.tensor_copy(out=u2, in_=mv2[:, :, :, 0])
            nc.vector.tensor_mul(out=q2, in0=mv2[:, :, :, 0], in1=mv2[:, :, :, 0])
            nc.vector.tensor_add(out=q2, in0=q2, in1=mv2[:, :, :, 1])
            nc.sync.dma_start(out=cc2_in[:, :], in_=uq2[:])
            nc.gpsimd.collective_compute(
                "AllReduce", mybir_.AluOpType.add, replica_groups=RG,
                ins=[cc2_in[:, :]], outs=[cc2_out[:, :]])
            s_all2 = stp.tile([P, 2 * D * M2], f32)
            nc.sync.dma_start(out=s_all2[:], in_=cc2_out[:, :])

            s2 = stp.tile([P, D * M2], f32)
            t2 = stp.tile([P, D * M2], f32)
            mean2 = stp.tile([P, D * M2], f32)
            var2 = stp.tile([P, D * M2], f32)
            nc.vector.tensor_scalar_mul(mean2[:], s_all2[:, 0:D * M2], 1.0 / NCORES)
            nc.vector.tensor_scalar_mul(var2[:], s_all2[:, D * M2:], 1.0 / NCORES)
            nc.vector.tensor_mul(out=s2[:], in0=mean2[:], in1=mean2[:])
            nc.vector.tensor_tensor(
                out=var2[:], in0=var2[:], in1=s2[:],
                op=mybir_.AluOpType.subtract)
            nc.scalar.activation(out=var2[:], in_=var2[:], func=AF.Sqrt,
                                 bias=eps_t[:], scale=1.0)
            nc.vector.reciprocal(out=var2[:], in_=var2[:])
            nc.vector.tensor_mul(out=s2[:], in0=g2c[:], in1=var2[:])
            nc.vector.tensor_mul(out=t2[:], in0=mean2[:], in1=s2[:])
            nc.vector.tensor_tensor(
                out=t2[:], in0=be2c[:], in1=t2[:], op=mybir_.AluOpType.subtract)

            # ---- Phase 4: a2 = relu(s2*h2+t2), dot W3, sigmoid ----
            for d in range(D):
                for nt in range(NT):
                    po = pso.tile([1, 512], f32, tag="po")
                    for m2 in range(M2):
                        h2r = h2p.tile([P, 512], f32, tag="h2r")
                        nc.sync.dma_start(
                            out=h2r[:],
                            in_=h2_dram[d, :, m2, nt * 512:(nt + 1) * 512])
                        dm = d * M2 + m2
                        a2 = outp.tile([P, 512], f32r, tag="a2")
                        nc.scalar.activation(
                            out=a2[:], in_=h2r[:], func=AF.Relu,
                            bias=t2[:, dm:dm + 1], scale=s2[:, dm:dm + 1])
                        nc.tensor.matmul(
                            out=po[:], lhsT=w3c[:, dm:dm + 1], rhs=a2[:],
                            start=(m2 == 0), stop=(m2 == M2 - 1))
                    sg = outp.tile([1, 512], f32, tag="sg")
                    nc.scalar.activation(
                        out=sg[:], in_=po[:], func=AF.Sigmoid,
                        bias=b3r[:, d:d + 1], scale=1.0)
                    nc.sync.dma_start(
                        out=out_d[d, nt * 512:(nt + 1) * 512], in_=sg[:])

    nc.compile()
    return nc


def kernel(**inputs):
    global _NC, LAST_EXEC_NS
    from concourse.bass_utils import run_bass_kernel_spmd

    feat_ids = np.asarray(inputs["feat_ids"])
    domain_id = np.asarray(inputs["domain_id"])
    emb_tables = np.ascontiguousarray(
        np.asarray(inputs["emb_tables"], dtype=np.float32))
    W1 = np.asarray(inputs["W1"], dtype=np.float32)
    b1 = np.asarray(inputs["b1"], dtype=np.float32)  # noqa: F841 (BN absorbs)
    g1 = np.asarray(inputs["g1"], dtype=np.float32)
    be1 = np.asarray(inputs["be1"], dtype=np.float32)
    W2 = np.asarray(inputs["W2"], dtype=np.float32)
    b2 = np.asarray(inputs["b2"], dtype=np.float32)  # noqa: F841 (BN absorbs)
    g2 = np.asarray(inputs["g2"], dtype=np.float32)
    be2 = np.asarray(inputs["be2"], dtype=np.float32)
    W3 = np.asarray(inputs["W3"], dtype=np.float32)
    b3 = np.asarray(inputs["b3"], dtype=np.float32)

    if _NC is None:
        _NC = _build()

    tab = np.ascontiguousarray(emb_tables.reshape(F * V, E))
    w1t = np.ascontiguousarray(W1.transpose(0, 2, 1))   # [D, IN, H1]
    w2t = np.ascontiguousarray(W2.transpose(0, 2, 1))   # [D, H1, H2]
    b3r = np.ascontiguousarray(b3.reshape(1, D))

    ids = feat_ids.astype(np.int64)
    in_maps = []
    for c in range(NCORES):
        idc = ids[c * BC:(c + 1) * BC]                   # [BC, F]
        g = idc.reshape(NBT, P, F).transpose(1, 0, 2).astype(np.int64)
        g = g + (np.arange(F, dtype=np.int64) * V)[None, None, :]
        gidx = np.ascontiguousarray(g.reshape(P, NBT * F).astype(np.int32))
        in_maps.append({
            "tab": tab, "gidx": gidx,
            "w1t": w1t, "w2t": w2t,
            "g1": g1, "be1": be1, "g2": g2, "be2": be2,
            "w3": W3, "b3": b3r,
        })

    res = run_bass_kernel_spmd(
        _NC, in_maps, core_ids=list(range(NCORES)), trace=bool(PROFILE))
    if PROFILE:
        LAST_EXEC_NS = res.exec_time_ns

    out_full = np.concatenate(
        [res.results[c]["out"] for c in range(NCORES)], axis=1)  # [D, B]
    final = out_full[domain_id.astype(np.int64), np.arange(B)]
    return final.astype(np.float32)
